# revision 2
# baseline (speedup 1.0000x reference)
"""Trainium2 Bass kernel v2 for 3-layer GAT (nn_MultiLayerGAT).

Structure (8 cores, dst-node sharding, 10 blocks of 128 dst nodes per core):
  Per layer L:
    phase A (local blocks only): xe row per node, bf16 [384]:
        [xp 0:256 | ald(f32 as 2xbf16) 256:272 | als(f32) 272:288 | junk]
      (layer 3, width 256: [xp 0:40 | als 40:42 | junk | ald 128:130 | junk])
      produced by one/two matmuls vs W_ext = [W | W@Ad | W@As] (bf16),
      assembled from f32 PSUM, DMA'd to cc_in; AllGather -> cc_out (xe).
    phase B per dst block:
      - main dma_gather by src: elem 384 units (768B), S idxs
      - al_d dma_gather by dst: QUAD-PACKED (4 edge slots of one cell share
        a dst), elem 128 units (256B), S/4 idxs
      - z = als[src]+ald[dst] (f32 via bitcast views), e=lrelu, ee=exp (bf16)
      - G[:, :, 0:256] *= ee (per-head broadcast); ee -> G cols 256:264
      - one-hot segment-sum: OH built in ONE DVE is_equal (bf16) from static
        dstc codes; CK matmuls psum += OH_j^T @ G_j  [128 dst, 264]
      - h = psum[0:256]/psum[256:264] + b; ELU -> bf16; PE-transpose -> lhsT
      - interleaved phase A of layer L+1 for this block
  Gathers round-robin over 4 SWDGE queues (desc-gen parallelism across
  GPSIMD Q7 pairs). Pads: gather idx 0, dstc=-1 => zero one-hot column.
"""

import numpy as np

N = 10000
E = 320000
IN = 128
HID = 32
HEADS = 8
HC = HEADS * HID          # 256
OUT = 40
NEG = 0.2

NPAD = 10240
NBLK_TOT = NPAD // 128    # 80
NCORES = 8
NB = NBLK_TOT // NCORES   # 10

ROW = 384                 # bf16 units per xe row, layers 1-2
ROW3 = 256                # layer 3
ALD_OFF = 256             # f32 ald at units [256:272]
ALS_OFF = 272             # f32 als at units [272:288]
ALS3_OFF = 40             # layer 3: f32 als at units [40:42]
ALD3_OFF = 128            # layer 3: f32 ald at units [128:130]


# ----------------------------------------------------------------------------
# host-side preprocessing
# ----------------------------------------------------------------------------

def build_w_ext(W, a_src, a_dst):
    """[W | W@Ad | W@As] columns (f32), matching the xe row layout order
    xp | ald | als. Returns [in, 256+8+8] or [in, 40+1+1] f32."""
    inn, hc = W.shape
    H, C = a_src.shape
    As = np.zeros((hc, H), np.float32)
    Ad = np.zeros((hc, H), np.float32)
    for h in range(H):
        As[h * C:(h + 1) * C, h] = a_src[h]
        Ad[h * C:(h + 1) * C, h] = a_dst[h]
    return np.concatenate([W, W @ Ad, W @ As], axis=1).astype(np.float32)


def wrap16(idx_flat):
    t16 = idx_flat.reshape(-1, 16).T.astype(np.int16)
    return np.tile(t16, (8, 1))


def preprocess(edge_index):
    """Quad-cell slot tables. Returns (CK, CQ, per-core dict arrays)."""
    src = np.concatenate([edge_index[0], np.arange(N, dtype=edge_index.dtype)])
    dst = np.concatenate([edge_index[1], np.arange(N, dtype=edge_index.dtype)])
    src = src.astype(np.int64)
    dst = dst.astype(np.int64)
    order = np.argsort(dst, kind="stable")
    ssrc, sdst = src[order], dst[order]

    # cells per block: (dcode, srcs[<=4])
    blk_starts = np.searchsorted(sdst, np.arange(0, NPAD + 1, 128))
    node_starts = np.searchsorted(sdst, np.arange(0, NPAD + 1))
    cells_all = []
    for b in range(NBLK_TOT):
        cells = []
        for d in range(128):
            nd = 128 * b + d
            lo, hi = node_starts[nd], node_starts[nd + 1]
            for i in range(lo, hi, 4):
                cells.append((d, ssrc[i:min(i + 4, hi)]))
        cells_all.append(cells)

    CQ = int(np.ceil(max(len(c) for c in cells_all) / 128))
    CK = 4 * CQ
    S = 128 * CK
    SQ = 128 * CQ

    # cc_out row remap for the 5-chunk AllGather layout (layers 2-3):
    # node n = 1280k + 128b + r -> row (b//2)*2048 + 256k + (b%2)*128 + r
    nid = np.arange(NPAD, dtype=np.int64)
    kk_, bb_, rr_ = nid // 1280, (nid // 128) % 10, nid % 128
    remap5 = (bb_ // 2) * (NCORES * 256) + kk_ * 256 + (bb_ % 2) * 128 + rr_

    import ml_dtypes
    cores = []
    for k in range(NCORES):
        gsrc = np.zeros((NB, S), np.int64)
        gald = np.zeros((NB, SQ), np.int64)
        dstc = np.full((NB, 128, CK), -1.0, np.float32)
        for bl in range(NB):
            b = k * NB + bl
            for i, (d, srcs) in enumerate(cells_all[b]):
                p, jq = i % 128, i // 128
                gald[bl, jq * 128 + p] = 128 * b + d
                for t, s in enumerate(srcs):
                    c = 4 * jq + t
                    gsrc[bl, c * 128 + p] = s
                    dstc[bl, p, c] = d
        cat = lambda a: np.concatenate([wrap16(a[bl]) for bl in range(NB)], axis=1)
        dc = np.ascontiguousarray(
            dstc.transpose(1, 0, 2).reshape(128, NB * CK))
        cores.append(dict(
            gsrc1=cat(gsrc), gald1=cat(gald),
            gsrc2=cat(remap5[gsrc]), gald2=cat(remap5[gald]),
            dstc=dc.astype(ml_dtypes.bfloat16)))
    return CK, CQ, cores


# ----------------------------------------------------------------------------
# bass program
# ----------------------------------------------------------------------------

def build_nc(CK, CQ):
    import os
    import concourse.bacc as bacc
    import concourse.mybir as mybir
    import concourse.tile as tile
    from concourse.library_config import mlp

    f32 = mybir.dt.float32
    bf16 = mybir.dt.bfloat16
    i16 = mybir.dt.int16
    Alu = mybir.AluOpType
    Act = mybir.ActivationFunctionType

    S = 128 * CK
    SQ = 128 * CQ

    nc = bacc.Bacc("TRN2", debug=False, num_swdge_queues=4)

    xT = nc.dram_tensor("xT", [IN, NB * 128], bf16, kind="ExternalInput")
    W1e = nc.dram_tensor("W1e", [IN, 272], bf16, kind="ExternalInput")
    W2e = nc.dram_tensor("W2e", [HC, 272], bf16, kind="ExternalInput")
    W3e = nc.dram_tensor("W3e", [HC, 42], bf16, kind="ExternalInput")
    gsrc1 = nc.dram_tensor("gsrc1", [128, NB * S // 16], i16, kind="ExternalInput")
    gald1 = nc.dram_tensor("gald1", [128, NB * SQ // 16], i16, kind="ExternalInput")
    gsrc2 = nc.dram_tensor("gsrc2", [128, NB * S // 16], i16, kind="ExternalInput")
    gald2 = nc.dram_tensor("gald2", [128, NB * SQ // 16], i16, kind="ExternalInput")
    dstc = nc.dram_tensor("dstc", [128, NB * CK], bf16, kind="ExternalInput")
    iota = nc.dram_tensor("iota", [128, CK * 128], bf16, kind="ExternalInput")
    ident = nc.dram_tensor("ident", [128, 128], bf16, kind="ExternalInput")
    b1r = nc.dram_tensor("b1r", [128, HC], f32, kind="ExternalInput")
    b2r = nc.dram_tensor("b2r", [128, HC], f32, kind="ExternalInput")
    b3r = nc.dram_tensor("b3r", [128, OUT], f32, kind="ExternalInput")

    out = nc.dram_tensor("out", [NB * 128, OUT], f32, kind="ExternalOutput")

    CBLK = 2 * 128                       # 256 rows per 2-block chunk
    cc_in1 = nc.dram_tensor("cc_in1", [NB * 128, ROW], bf16)
    cc_out1 = nc.dram_tensor("cc_out1", [NPAD, ROW], bf16, addr_space="Shared")
    cc_in2 = [nc.dram_tensor(f"cc_in2{c}", [CBLK, ROW], bf16) for c in range(5)]
    cc_out2 = nc.dram_tensor("cc_out2", [NPAD, ROW], bf16, addr_space="Shared")
    cc_in3 = [nc.dram_tensor(f"cc_in3{c}", [CBLK, ROW3], bf16) for c in range(5)]
    cc_out3 = nc.dram_tensor("cc_out3", [NPAD, ROW3], bf16, addr_space="Shared")

    qctr = [0]

    def next_q():
        q = qctr[0] % 4
        qctr[0] += 1
        return q

    with tile.TileContext(nc) as tc:
        nc.gpsimd.load_library(mlp)
        with tc.tile_pool(name="const", bufs=1) as cpool, \
             tc.tile_pool(name="g", bufs=3) as gpool, \
             tc.tile_pool(name="dq", bufs=3) as dpool, \
             tc.tile_pool(name="oh", bufs=3) as ohpool, \
             tc.tile_pool(name="sm", bufs=3) as spool, \
             tc.tile_pool(name="post", bufs=3) as ppool, \
             tc.tile_pool(name="psA", bufs=2, space="PSUM") as psA, \
             tc.tile_pool(name="psB", bufs=2, space="PSUM") as psB, \
             tc.tile_pool(name="psT", bufs=2, space="PSUM") as psT:

            # constants
            gsrc1_t = cpool.tile([128, NB * S // 16], i16, tag="gsrc1")
            nc.sync.dma_start(gsrc1_t[:], gsrc1[:])
            gald1_t = cpool.tile([128, NB * SQ // 16], i16, tag="gald1")
            nc.sync.dma_start(gald1_t[:], gald1[:])
            gsrc2_t = cpool.tile([128, NB * S // 16], i16, tag="gsrc2")
            nc.sync.dma_start(gsrc2_t[:], gsrc2[:])
            gald2_t = cpool.tile([128, NB * SQ // 16], i16, tag="gald2")
            nc.sync.dma_start(gald2_t[:], gald2[:])
            ident_t = cpool.tile([128, 128], bf16, tag="ident")
            nc.sync.dma_start(ident_t[:], ident[:])
            b1_t = cpool.tile([128, HC], f32, tag="b1")
            nc.sync.dma_start(b1_t[:], b1r[:])
            b2_t = cpool.tile([128, HC], f32, tag="b2")
            nc.sync.dma_start(b2_t[:], b2r[:])
            b3_t = cpool.tile([128, OUT], f32, tag="b3")
            nc.sync.dma_start(b3_t[:], b3r[:])
            xT_t = cpool.tile([128, NB * 128], bf16, tag="xT")
            nc.sync.dma_start(xT_t[:], xT[:])
            w1_t = cpool.tile([128, 272], bf16, tag="w1")
            nc.sync.dma_start(w1_t[:], W1e[:])
            w2_t = cpool.tile([128, 2, 272], bf16, tag="w2")
            for kk in range(2):
                nc.sync.dma_start(w2_t[:, kk, :], W2e[kk * 128:(kk + 1) * 128, :])
            w3_t = cpool.tile([128, 2, 42], bf16, tag="w3")
            for kk in range(2):
                nc.sync.dma_start(w3_t[:, kk, :], W3e[kk * 128:(kk + 1) * 128, :])
            dstc_t = cpool.tile([128, NB * CK], bf16, tag="dstc")
            nc.sync.dma_start(dstc_t[:], dstc[:])
            iota_t = cpool.tile([128, CK * 128], bf16, tag="iota")
            nc.sync.dma_start(iota_t[:], iota[:])
            zer_t = cpool.tile([128, HC], f32, tag="zer")
            nc.vector.memset(zer_t[:], 0.0)
            mon_t = cpool.tile([128, HC], f32, tag="mon")
            nc.vector.memset(mon_t[:], -1.0)

            def phase_a_block(layer, b, lhsT_parts):
                """xe row assembly for own block b of `layer` -> cc_in."""
                if layer == 1:
                    wt, ncols, cc = w1_t, 272, cc_in1
                elif layer == 2:
                    wt, ncols, cc = w2_t, 272, cc_in2
                else:
                    wt, ncols, cc = w3_t, 42, cc_in3
                row = ROW if layer < 3 else ROW3
                ps = psA.tile([128, 272], f32, tag="psa")
                nk = len(lhsT_parts)
                for kk, lhs in enumerate(lhsT_parts):
                    nc.tensor.matmul(ps[:, 0:ncols], lhs,
                                     wt[:] if layer == 1 else wt[:, kk, :],
                                     start=(kk == 0), stop=(kk == nk - 1))
                xa = ppool.tile([128, row], bf16, tag=f"xa{layer}")
                if layer < 3:
                    nc.vector.tensor_copy(xa[:, 0:256], ps[:, 0:256])
                    nc.vector.tensor_copy(
                        xa[:, ALD_OFF:ALD_OFF + 16].bitcast(f32), ps[:, 256:264])
                    nc.vector.tensor_copy(
                        xa[:, ALS_OFF:ALS_OFF + 16].bitcast(f32), ps[:, 264:272])
                else:
                    nc.vector.tensor_copy(xa[:, 0:40], ps[:, 0:40])
                    nc.vector.tensor_copy(
                        xa[:, ALD3_OFF:ALD3_OFF + 2].bitcast(f32), ps[:, 40:41])
                    nc.vector.tensor_copy(
                        xa[:, ALS3_OFF:ALS3_OFF + 2].bitcast(f32), ps[:, 41:42])
                if layer == 1:
                    nc.sync.dma_start(cc[b * 128:(b + 1) * 128, :], xa[:])
                else:
                    nc.sync.dma_start(
                        cc[b // 2][(b % 2) * 128:(b % 2) * 128 + 128, :], xa[:])

            def phase_b(layer, xe, b_t):
                """aggregation over NB blocks; interleaves next layer's phase A."""
                nh = HEADS if layer < 3 else 1
                fe = HC if layer < 3 else OUT
                row = ROW if layer < 3 else ROW3
                grow = row if layer < 3 else 128
                aldo = ALD_OFF if layer < 3 else ALD3_OFF
                also = ALS_OFF if layer < 3 else ALS3_OFF
                sfx = "12" if layer < 3 else "3"
                gs_t = gsrc1_t if layer == 1 else gsrc2_t
                ga_t = gald1_t if layer == 1 else gald2_t
                def head(b):
                    # gathers: main split into halves on rotating queues so
                    # the 4 GPSIMD Q7 pairs stay load-balanced
                    g_t = gpool.tile([128, CK, grow], bf16, tag="G12" if layer < 3 else "G3")
                    H = S // 2
                    for half in range(2):
                        nc.gpsimd.dma_gather(
                            g_t[:, half * (CK // 2):(half + 1) * (CK // 2), :],
                            xe[:, 0:grow],
                            gs_t[:, (b * S + half * H) // 16:
                                 (b * S + (half + 1) * H) // 16],
                            H, H, grow,
                            elem_step=row, single_packet=False,
                            queue_num=next_q())
                    d_t = dpool.tile([128, CQ, 128], bf16, tag="D")
                    nc.gpsimd.dma_gather(
                        d_t[:], xe[:, aldo:aldo + 128],
                        ga_t[:, b * SQ // 16:(b + 1) * SQ // 16], SQ, SQ, 128,
                        elem_step=row, single_packet=False, queue_num=next_q())

                    # z = als + ald (f32) in [p, j, h, t] layout (quad axis
                    # trailing so ald broadcasts by append); e=lrelu; ee=exp
                    z_t = spool.tile([128, CQ, nh, 4], f32, tag="z" + sfx)
                    als_v = (g_t[:, :, also:also + 2 * nh].bitcast(f32)
                             .rearrange("p (j t) h -> p j h t", t=4))
                    ald_v = (d_t[:, :, 0:2 * nh].bitcast(f32)
                             .to_broadcast([128, CQ, nh, 4]))
                    nc.vector.tensor_tensor(z_t[:], als_v, ald_v, Alu.add)
                    zf = z_t[:].rearrange("p j h t -> p (j h t)")
                    nc.vector.scalar_tensor_tensor(
                        zf, zf, NEG, zf, Alu.mult, Alu.max)
                    eeq = spool.tile([128, CQ, nh, 4], bf16, tag="eeq" + sfx)
                    nc.scalar.activation(
                        eeq[:].rearrange("p j h t -> p (j h t)"), zf, Act.Exp)
                    # ee in edge-slot order [p, c=(j t), 1, h]
                    ee_t = spool.tile([128, CK, 1, nh], bf16, tag="ee" + sfx)
                    nc.vector.tensor_copy(
                        ee_t[:].rearrange("p (j t) u h -> p j t (u h)", t=4),
                        eeq[:].rearrange("p j h t -> p j t h"))
                    ee_c = ee_t[:].rearrange("p c u h -> p c (u h)")

                    # scale gathered features, stash ee as denominator cols
                    if layer < 3:
                        # xp stored (z h)-major so every operand is stride-1
                        # innermost -> DVE 2x/4x mode
                        nc.vector.tensor_tensor(
                            g_t[:, :, 0:fe].rearrange("p c (z h) -> p c z h", h=nh),
                            g_t[:, :, 0:fe].rearrange("p c (z h) -> p c z h", h=nh),
                            ee_t[:].to_broadcast([128, CK, HID, nh]),
                            Alu.mult)
                        nc.vector.tensor_copy(g_t[:, :, fe:fe + nh], ee_c)
                    else:
                        nc.vector.tensor_tensor(
                            g_t[:, :, 0:fe],
                            g_t[:, :, 0:fe],
                            ee_c.to_broadcast([128, CK, fe]),
                            Alu.mult)
                        nc.vector.tensor_copy(g_t[:, :, fe:fe + nh], ee_c)

                    # one-hot tiles: ONE fast tensor_tensor is_equal
                    oh_t = ohpool.tile([128, CK, 128], bf16, tag="OH")
                    nc.vector.tensor_tensor(
                        oh_t[:],
                        dstc_t[:, b * CK:(b + 1) * CK].to_broadcast(
                            [128, CK, 128]),
                        iota_t[:].rearrange("p (c q) -> p c q", q=128),
                        Alu.is_equal)

                    rw = fe + nh
                    ps = psB.tile([128, rw], f32, tag="agg" + sfx)
                    for j in range(CK):
                        nc.tensor.matmul(
                            ps[:], oh_t[:, j, :], g_t[:, j, 0:rw],
                            start=(j == 0), stop=(j == CK - 1))
                    return ps

                def tail(b, ps):
                    r_t = spool.tile([128, 1, nh], f32, tag="recip" + sfx)
                    r_c = r_t[:].rearrange("p u h -> p (u h)")
                    nc.vector.tensor_scalar(
                        r_c, ps[:, fe:fe + nh], 1e-16, None, Alu.add)
                    nc.vector.reciprocal(r_c, r_c)
                    h_t = ppool.tile([128, fe], f32, tag="H" + sfx)
                    if layer < 3:
                        nc.vector.tensor_tensor(
                            h_t[:].rearrange("p (z h) -> p z h", h=nh),
                            ps[:, 0:fe].rearrange("p (z h) -> p z h", h=nh),
                            r_t[:].to_broadcast([128, HID, nh]),
                            Alu.mult)
                        nc.vector.tensor_tensor(h_t[:], h_t[:], b_t[:], Alu.add)
                        # ELU -> bf16 (avoid slow DVE tensor_scalar forms)
                        t2 = ppool.tile([128, fe], f32, tag="elu")
                        nc.vector.tensor_tensor(t2[:], h_t[:], zer_t[:], Alu.min)
                        nc.scalar.activation(t2[:], t2[:], Act.Exp)
                        nc.vector.scalar_tensor_tensor(
                            h_t[:], h_t[:], 0.0, t2[:], Alu.max, Alu.add)
                        hb = ppool.tile([128, fe], bf16, tag="hb")
                        nc.vector.tensor_tensor(hb[:], h_t[:], mon_t[:], Alu.add)
                        # transpose for next layer's lhsT
                        ht = ppool.tile([128, 2, 128], bf16, tag="ht")
                        for half in range(2):
                            pt = psT.tile([128, 128], bf16, tag="tr")
                            nc.tensor.transpose(
                                pt[:], hb[:, half * 128:(half + 1) * 128],
                                ident_t[:])
                            nc.vector.tensor_copy(ht[:, half, :], pt[:])
                        phase_a_block(layer + 1, b, [ht[:, 0, :], ht[:, 1, :]])
                        nin, nout = ((cc_in2, cc_out2) if layer == 1
                                     else (cc_in3, cc_out3))
                        if b % 2 == 1:
                            allgather_chunk(nin, nout, b // 2)
                    else:
                        nc.vector.tensor_tensor(
                            h_t[:], ps[:, 0:fe],
                            r_t[:, 0, :].to_broadcast([128, fe]),
                            Alu.mult)
                        nc.vector.tensor_tensor(h_t[:], h_t[:], b_t[:], Alu.add)
                        m_t = spool.tile([128, 1], f32, tag="m")
                        nc.vector.tensor_reduce(
                            m_t[:], h_t[:], mybir.AxisListType.X, Alu.max)
                        nc.vector.tensor_tensor(
                            h_t[:], h_t[:], m_t[:].to_broadcast([128, fe]),
                            Alu.subtract)
                        x_t = ppool.tile([128, fe], f32, tag="exps")
                        s_t = spool.tile([128, 1], f32, tag="s")
                        nc.scalar.activation(
                            x_t[:], h_t[:], Act.Exp, accum_out=s_t[:])
                        l_t = spool.tile([128, 1], f32, tag="l")
                        nc.scalar.activation(l_t[:], s_t[:], Act.Ln)
                        nc.vector.tensor_tensor(
                            h_t[:], h_t[:], l_t[:].to_broadcast([128, fe]),
                            Alu.subtract)
                        nc.sync.dma_start(out[b * 128:(b + 1) * 128, :], h_t[:])

                prev = None
                for b in range(NB):
                    ps = head(b)
                    if prev is not None:
                        tail(*prev)
                    prev = (b, ps)
                tail(*prev)

            def allgather_chunk(cin, cout, chunk):
                creg = NCORES * CBLK                 # 2048
                nc.gpsimd.collective_compute(
                    "AllGather", mybir.AluOpType.bypass,
                    replica_groups=[list(range(NCORES))],
                    ins=[cin[chunk].ap().opt()],
                    outs=[cout[chunk * creg:(chunk + 1) * creg, :].opt()])

            mode = os.environ.get("GAT_MODE", "full")
            # layer 1 phase A, one collective (natural row order)
            for b in range(NB):
                phase_a_block(1, b, [xT_t[:, b * 128:(b + 1) * 128]])
            if mode != "a":
                nc.gpsimd.collective_compute(
                    "AllGather", mybir.AluOpType.bypass,
                    replica_groups=[list(range(NCORES))],
                    ins=[cc_in1.ap().opt()], outs=[cc_out1.ap().opt()])
            if mode == "a":
                # dump first 40 xp cols of own cc_in1 rows for debugging
                for b in range(NB):
                    td = ppool.tile([128, OUT], bf16, tag="dumpb")
                    nc.sync.dma_start(td[:], cc_in1[b * 128:(b + 1) * 128, 0:OUT])
                    tf = ppool.tile([128, OUT], f32, tag="dumpf")
                    nc.vector.tensor_copy(tf[:], td[:])
                    nc.sync.dma_start(out[b * 128:(b + 1) * 128, :], tf[:])
            else:
                phase_b(1, cc_out1, b1_t)
                phase_b(2, cc_out2, b2_t)
                phase_b(3, cc_out3, b3_t)

    nc.compile()
    return nc


# ----------------------------------------------------------------------------
# entry point
# ----------------------------------------------------------------------------

LAST_EXEC_NS = None


def kernel(**inputs):
    import os
    import ml_dtypes
    from concourse.bass_utils import run_bass_kernel_spmd
    global LAST_EXEC_NS

    bf = ml_dtypes.bfloat16
    x = np.asarray(inputs["x"], np.float32)
    ei = np.asarray(inputs["edge_index"])
    CK, CQ, cores = preprocess(ei)

    # (z h)-major feature order for layers 1-2 hidden features
    g_ = np.arange(HC)
    perm = (g_ % HEADS) * HID + g_ // HEADS   # new col g <- old col perm[g]

    xp = np.zeros((NPAD, IN), np.float32)
    xp[0:N] = x
    W1en = build_w_ext(np.asarray(inputs["W1"], np.float32),
                       np.asarray(inputs["a_src1"], np.float32),
                       np.asarray(inputs["a_dst1"], np.float32))
    W2en = build_w_ext(np.asarray(inputs["W2"], np.float32),
                       np.asarray(inputs["a_src2"], np.float32),
                       np.asarray(inputs["a_dst2"], np.float32))
    W3en = build_w_ext(np.asarray(inputs["W3"], np.float32),
                       np.asarray(inputs["a_src3"], np.float32),
                       np.asarray(inputs["a_dst3"], np.float32))
    W1en[:, 0:HC] = W1en[:, perm]
    W2en[:, 0:HC] = W2en[:, perm]
    W2en = W2en[perm, :]
    W3en = W3en[perm, :]
    W1en, W2en, W3en = (w.astype(bf) for w in (W1en, W2en, W3en))
    iota_n = np.tile(np.arange(128, dtype=np.float32), (128, CK)).astype(bf)
    ident_n = np.eye(128, dtype=np.float32).astype(bf)
    b1n = np.tile(np.asarray(inputs["b1"], np.float32)[perm], (128, 1))
    b2n = np.tile(np.asarray(inputs["b2"], np.float32)[perm], (128, 1))
    b3n = np.tile(np.asarray(inputs["b3"], np.float32), (128, 1))

    nc = build_nc(CK, CQ)
    in_maps = []
    for k in range(NCORES):
        xTme = np.ascontiguousarray(
            xp[k * NB * 128:(k + 1) * NB * 128].T).astype(bf)
        in_maps.append({
            "xT": xTme, "W1e": W1en, "W2e": W2en, "W3e": W3en,
            "gsrc1": cores[k]["gsrc1"], "gald1": cores[k]["gald1"],
            "gsrc2": cores[k]["gsrc2"], "gald2": cores[k]["gald2"],
            "dstc": cores[k]["dstc"], "iota": iota_n, "ident": ident_n,
            "b1r": b1n, "b2r": b2n, "b3r": b3n,
        })
    trace = bool(int(os.environ.get("GAT_TRACE", "0")))
    res = run_bass_kernel_spmd(nc, in_maps, list(range(NCORES)), trace=trace)
    LAST_EXEC_NS = res.exec_time_ns
    full = np.concatenate([res.results[k]["out"] for k in range(NCORES)], axis=0)
    return full[0:N].astype(np.float32)
